# revision 85
# baseline (speedup 1.0000x reference)
import numpy as np

# nn_Attention: B=256, N=65, DIM=1024, HEADS=16, DH=64 across 8 cores (32 batches/core)
B, N, DIM, HEADS, DH = 256, 65, 1024, 16, 64
NCORES = 8
BPC = B // NCORES            # 32 batches per core
TOK = BPC * N                # 2080 tokens per core
BB = 4                       # batches per block
NBLK = BPC // BB             # 8 blocks
TB = BB * N                  # 260 tokens per block
BN_EPS = 1e-5
ROWW = 68                    # vph row width: [1, 0, v(64), 0, dead]
NROW = 18                    # 16 head rows + 2 pad rows


def _build(nc_mod, mybir, bass):
    import os
    NBLKB = int(os.environ.get("K_BLOCKS", str(NBLK)))
    SKIP = set(os.environ.get("K_SKIP", "").split(","))
    f32 = mybir.dt.float32
    f32r = mybir.dt.float32r
    bf16 = mybir.dt.bfloat16
    Act = mybir.ActivationFunctionType
    Alu = mybir.AluOpType
    from concourse.tile import TileContext

    nc = nc_mod
    xt = nc.declare_dram_parameter("xt", [DIM, TOK], f32r, isOutput=False)
    wqkvt = nc.declare_dram_parameter("wqkvt", [DIM, 3 * DIM], f32r, isOutput=False)
    woutt = nc.declare_dram_parameter("woutt", [DIM, DIM], bf16, isOutput=False)
    wconvt = nc.declare_dram_parameter("wconvt", [9, N, N], bf16, isOutput=False)
    bqk = nc.declare_dram_parameter("bqk", [128, 8], f32, isOutput=False)
    id65 = nc.declare_dram_parameter("id65", [N, N], bf16, isOutput=False)
    id128 = nc.declare_dram_parameter("id128", [128, 128], bf16, isOutput=False)
    out = nc.declare_dram_parameter("out", [DIM, TOK], f32, isOutput=True)

    scale = float(DIM) ** -0.5

    from contextlib import ExitStack
    with TileContext(nc) as tc:
        with ExitStack() as es:
            P = lambda *a, **k: es.enter_context(tc.tile_pool(*a, **k))
            cp = P(name="consts", bufs=1)
            xtp = P(name="xtp", bufs=2)
            qtp = P(name="qtp", bufs=2)
            ktp = P(name="ktp", bufs=2)
            vpp = P(name="vpp", bufs=8)
            vtp = P(name="vtp", bufs=2)
            expp = P(name="exps", bufs=3)
            recp = P(name="recips", bufs=3)
            resp = P(name="resp", bufs=3)
            rtp = P(name="rtp", bufs=3)
            osbp = P(name="osb", bufs=4)
            pbig = P(name="pbig", bufs=2, space="PSUM")
            patt_a = P(name="patt_a", bufs=1, space="PSUM")
            patt_b = P(name="patt_b", bufs=1, space="PSUM")
            pcv = P(name="pconv", bufs=2, space="PSUM")
            ptr = P(name="ptr", bufs=2, space="PSUM")

            # ---- resident constants ----
            # small constants first, then q/k weight column-tiles in the order
            # the first block consumes them, so PE can start ~1.6MB in
            bqk_sb = cp.tile([128, 8], f32, tag="bqk")
            nc.sync.dma_start(out=bqk_sb[:], in_=bqk[:])
            id_sb = cp.tile([N, N], bf16, tag="id")
            nc.sync.dma_start(out=id_sb[:], in_=id65[:])
            id128_sb = cp.tile([128, 128], bf16, tag="id128")
            nc.sync.dma_start(out=id128_sb[:], in_=id128[:])
            wconv_sb = cp.tile([N, 9 * 66], bf16, tag="wconv")
            wconv = wconv_sb[:].rearrange("i (t o) -> i t o", t=9)
            nc.sync.dma_start(
                out=wconv[:, :, 0:N],
                in_=wconvt[:].rearrange("t i o -> i t o"),
            )

            def dma_xt(b):
                t0b = b * TB
                xt_sb = xtp.tile([128, 8 * TB], f32r, tag="xt")
                nc.sync.dma_start(
                    out=xt_sb[:].rearrange("p (a n) -> p a n", a=8),
                    in_=xt[:].rearrange("(a p) n -> p a n", p=128)[:, :, t0b:t0b + TB],
                )
                return xt_sb

            wqkv_sb = cp.tile([128, 8 * 3 * DIM], f32r, tag="wqkv")
            wqkv = wqkv_sb[:].rearrange("p (a n) -> p a n", a=8)

            def dma_wqkv_col(m):
                nc.sync.dma_start(
                    out=wqkv[:, :, m * 128:(m + 1) * 128],
                    in_=wqkvt[:, m * 128:(m + 1) * 128].rearrange(
                        "(k p) c -> p k c", p=128),
                )

            # block-0 xt split per ki slice, first q column right after
            # slice 0, so the very first matmul group starts ~2us in
            xt_cur = xtp.tile([128, 8 * TB], f32r, tag="xt")
            xt0v = xt_cur[:].rearrange("p (a n) -> p a n", a=8)
            xt_src = xt[:].rearrange("(a p) n -> p a n", p=128)
            nc.sync.dma_start(out=xt0v[:, 0, :], in_=xt_src[:, 0, 0:TB])
            dma_wqkv_col(0)              # q m-tile 0
            for ki in range(1, 8):
                nc.sync.dma_start(out=xt0v[:, ki, :], in_=xt_src[:, ki, 0:TB])
            dma_wqkv_col(8)              # k m-tile 0
            dma_wqkv_col(16)             # v m-tile 0
            for a in range(1, 8):
                dma_wqkv_col(a)          # q m-tile a
                dma_wqkv_col(8 + a)      # k m-tile a
                dma_wqkv_col(16 + a)     # v m-tile a

            wout_sb = cp.tile([128, 8 * DIM], bf16, tag="wout")
            wout = wout_sb[:].rearrange("p (a n) -> p a n", a=8)
            for ki in range(8):
                nc.sync.dma_start(
                    out=wout[:, ki, :],
                    in_=woutt[ki * 128:(ki + 1) * 128, :],
                )

            for blk in range(NBLKB):
                t0 = blk * TB
                xtv = xt_cur[:].rearrange("p (a n) -> p a n", a=8)
                if blk + 1 < NBLKB:
                    xt_next = dma_xt(blk + 1)

                # ---- Q^T, K^T projections -> bf16 [feat-tile, a, tok] ----
                qt_sb = qtp.tile([128, 8 * TB], bf16, tag="qt")
                qtv = qt_sb[:].rearrange("p (a n) -> p a n", a=8)
                kt_sb = ktp.tile([128, 8 * TB], bf16, tag="kt")
                ktv = kt_sb[:].rearrange("p (a n) -> p a n", a=8)
                vt_sb = vtp.tile([128, 8 * TB], bf16, tag="vt")
                vtv = vt_sb[:].rearrange("p (a n) -> p a n", a=8)
                for a in range(8):
                    pq = pbig.tile([128, TB], f32, tag="big")
                    for ki in range(8):
                        nc.tensor.matmul(
                            pq[:], wqkv[:, ki, a * 128:(a + 1) * 128],
                            xtv[:, ki, :], start=(ki == 0), stop=(ki == 7),
                        )
                    nc.scalar.copy(qtv[:, a, :], pq[:])
                    pk = pbig.tile([128, TB], f32, tag="big")
                    for ki in range(8):
                        nc.tensor.matmul(
                            pk[:], wqkv[:, ki, DIM + a * 128:DIM + (a + 1) * 128],
                            xtv[:, ki, :], start=(ki == 0), stop=(ki == 7),
                        )
                    nc.vector.tensor_scalar_add(
                        ktv[:, a, :], pk[:], bqk_sb[:, a:a + 1])
                    pvt = pbig.tile([128, TB], f32, tag="big")
                    for ki in range(8):
                        nc.tensor.matmul(
                            pvt[:], wqkv[:, ki, 2 * DIM + a * 128:2 * DIM + (a + 1) * 128],
                            xtv[:, ki, :], start=(ki == 0), stop=(ki == 7),
                        )
                    if a % 2 == 0:
                        nc.vector.tensor_copy(vtv[:, a, :], pvt[:])
                    else:
                        nc.scalar.copy(vtv[:, a, :], pvt[:])

                rt_sb = rtp.tile([128, 8 * TB], bf16, tag="rt")
                rtv = rt_sb[:].rearrange("p (k n) -> p k n", k=8)
                if "trans" in SKIP:
                    nc.vector.memset(rt_sb[:], 0.0)

                # ---- v images for all 4 batches: [1, 0, v(64), 0, _] rows ----
                vphs = []
                for bi in range(BB):
                    toff = bi * N
                    vp = vpp.tile([N, NROW * ROWW], bf16, tag="vp")
                    vph = vp[:].rearrange("p (r c) -> p r c", c=ROWW)
                    nc.gpsimd.memset(vph[:, 0:1, :], 0.0)
                    nc.gpsimd.memset(vph[:, NROW - 1:NROW, :], 0.0)
                    nc.gpsimd.memset(vph[:, 1:NROW - 1, 0:1], 1.0)
                    nc.gpsimd.memset(vph[:, 1:NROW - 1, 1:2], 0.0)
                    nc.gpsimd.memset(vph[:, 1:NROW - 1, 66:67], 0.0)
                    if "v" in SKIP:
                        nc.vector.memset(vph[:, 1:NROW - 1, 2:66], 0.0)
                    for half in range(0 if "v" not in SKIP else 2, 2):
                        pt2 = ptr.tile([N, 512], bf16, tag="pt")
                        for j in range(4):
                            a = 4 * half + j
                            nc.tensor.transpose(
                                pt2[:, j * 128:(j + 1) * 128],
                                vtv[:, a, toff:toff + N], id128_sb[:])
                        dst = vph[:, 1 + 8 * half:9 + 8 * half, 2:66]
                        srcv = pt2[:].rearrange("p (r c) -> p r c", c=64)
                        if half == 0:
                            nc.vector.tensor_copy(dst, srcv)
                        else:
                            nc.scalar.copy(dst, srcv)
                    vphs.append(vph)

                for bi in range(BB):
                    toff = bi * N
                    gtok = t0 + toff
                    vph = vphs[bi]

                    res = resp.tile([N, DIM], bf16, tag="res")
                    o_dummy = None
                    if "norm" in SKIP:
                        o_dummy = resp.tile([N, 4], f32, tag="odum")
                    if "att" in SKIP:
                        nc.vector.memset(res[:], 0.0)
                    # ---- attention: 4-head groups, bf16 ----
                    for g in range(0 if "att" not in SKIP else 4, 4):
                        # one row-group (poff) per psum bank: concurrent
                        # row-tiled matmuls must not share a PSUM bank
                        parity, ahalf = g % 2, g // 2
                        poff = parity * 64
                        heads = [2 * a + parity for a in
                                 range(ahalf * 4, ahalf * 4 + 4)]
                        patt = patt_a if g % 2 == 0 else patt_b
                        pd = patt.tile([N, 4 * 66], f32, tag="att")
                        pdg = pd[:].rearrange("p (h c) -> p h c", c=66)
                        for hh, h in enumerate(heads):
                            a = h // 2
                            nc.tensor.matmul(
                                pdg[:, hh, 0:N],
                                ktv[poff:poff + 64, a, toff:toff + N],
                                qtv[poff:poff + 64, a, toff:toff + N],
                                start=True, stop=True,
                            )
                        ex = expp.tile([N, 4 * 66], bf16, tag="ex")
                        exg = ex[:].rearrange("p (h c) -> p h c", c=66)
                        nc.scalar.activation(
                            exg[:, :, 0:N], pdg[:, :, 0:N], Act.Exp, scale=scale)
                        if "av" in SKIP:
                            if g == 0:
                                nc.vector.memset(res[:], 0.0)
                            continue
                        po = patt.tile([N, 4 * 68], f32, tag="att")
                        pog = po[:].rearrange("p (h c) -> p h c", c=68)
                        for hh, h in enumerate(heads):
                            nc.tensor.matmul(
                                pog[:, hh, 0:67],
                                exg[:, hh, 0:N],
                                vph[:, 1 + h, 0:67],
                                start=True, stop=True,
                            )
                        if "norm" in SKIP:
                            if g == 0:
                                nc.vector.memset(res[:], 0.0)
                            nc.vector.tensor_copy(
                                o_dummy[:, g:g + 1], pog[:, 0, 0:1])
                            continue
                        rc = recp.tile([N, 4], f32, tag="rc")
                        nc.vector.reciprocal(
                            rc[:], pog[:, :, 0:1].rearrange("p h c -> p (h c)"))
                        # res head slices for this group: head stride 2 (parity)
                        resv = res[:].rearrange(
                            "p (a q c) -> p a q c", a=8, c=DH)[
                            :, ahalf * 4:ahalf * 4 + 4, parity:parity + 1, :]
                        pov = pog[:, :, 2:66].rearrange(
                            "p h (u c) -> p h u c", u=1)
                        rcb = rc[:].rearrange("p (h u) -> p h u", u=1).rearrange(
                            "p h (u c) -> p h u c", u=1).broadcast_to(
                            [N, 4, 1, DH])
                        nc.vector.tensor_tensor(
                            resv, pov, rcb, op=Alu.mult)

                    # ---- conv 3x3 SAME (bf16, bn-scale folded into weights) ----
                    for half in range(0 if "conv" not in SKIP else 2, 2):
                        y1 = 1 + 8 * half
                        pc = pcv.tile([N, 512], f32, tag="pc")
                        pcy = pc[:].rearrange("p (y x) -> p y x", x=64)
                        for t in range(9):
                            dy, dx = t // 3 - 1, t % 3 - 1
                            nc.tensor.matmul(
                                pcy[:],
                                wconv[:, t, 0:N],
                                vph[:, y1 + dy:y1 + dy + 8, 2 + dx:2 + dx + 64],
                                start=(t == 0), stop=(t == 8),
                            )
                        nc.vector.tensor_add(
                            res[:, half * 512:(half + 1) * 512],
                            res[:, half * 512:(half + 1) * 512],
                            pc[:])

                    # ---- transpose res -> rtT [dim-tile, ki, tok] ----
                    # 4 transposes per bank (68-col stride keeps dsts 8B-aligned)
                    for q4 in range(0 if "trans" not in SKIP else 2, 2):
                        pt = ptr.tile([128, 4 * 68], bf16, tag="pt")
                        ptg = pt[:].rearrange("p (j c) -> p j c", c=68)
                        for j in range(4):
                            c8 = q4 * 4 + j
                            nc.tensor.transpose(
                                ptg[:, j, 0:N],
                                res[:, c8 * 128:(c8 + 1) * 128], id_sb[:])
                        dstr = rtv[:, q4 * 4:q4 * 4 + 4, toff:toff + N]
                        if q4 == 0:
                            nc.vector.tensor_copy(dstr, ptg[:, :, 0:N])
                        else:
                            nc.scalar.copy(dstr, ptg[:, :, 0:N])

                # ---- final projection, transposed: out^T[feat, tok] ----
                for ft in range(8):
                    pf = pbig.tile([128, TB], f32, tag="big")
                    for ki in range(8):
                        nc.tensor.matmul(
                            pf[:],
                            wout[:, ki, ft * 128:(ft + 1) * 128],
                            rtv[:, ki, :],
                            start=(ki == 0), stop=(ki == 7),
                        )
                    ob = osbp.tile([128, TB], f32, tag="ob")
                    if ft % 2 == 0:
                        nc.vector.tensor_copy(ob[:], pf[:])
                    else:
                        nc.scalar.copy(ob[:], pf[:])
                    nc.sync.dma_start(
                        out=out[ft * 128:(ft + 1) * 128, t0:t0 + TB], in_=ob[:])
                if blk + 1 < NBLKB:
                    xt_cur = xt_next
    return nc


def _prepare_inputs(inputs):
    import ml_dtypes
    bf = ml_dtypes.bfloat16
    x = np.asarray(inputs["x"], np.float32)
    w_qkv = np.asarray(inputs["w_qkv"], np.float32)
    b_qkv = np.asarray(inputs["b_qkv"], np.float32)
    w_out = np.asarray(inputs["w_out"], np.float32)
    conv_w = np.asarray(inputs["conv_w"], np.float32)
    bn_gamma = np.asarray(inputs["bn_gamma"], np.float32)
    bn_var = np.asarray(inputs["bn_var"], np.float32)

    xt_all = np.ascontiguousarray(x.reshape(B * N, DIM).T)      # [1024, 16640]
    wqkvt = np.ascontiguousarray(w_qkv.T)
    woutt = np.ascontiguousarray(w_out.T.astype(bf))
    s = bn_gamma / np.sqrt(bn_var + BN_EPS)
    wc = conv_w.transpose(2, 3, 1, 0).reshape(9, N, N)          # [t, i, o]
    wconvt = np.ascontiguousarray((wc * s[None, None, :]).astype(bf))
    bqkm = np.ascontiguousarray(b_qkv[DIM:2 * DIM].reshape(8, 128).T)
    identm = np.eye(N, dtype=bf)
    ident128 = np.eye(128, dtype=bf)

    in_maps = []
    for c in range(NCORES):
        in_maps.append({
            "xt": np.ascontiguousarray(xt_all[:, c * TOK:(c + 1) * TOK]),
            "wqkvt": wqkvt, "woutt": woutt, "wconvt": wconvt,
            "bqk": bqkm, "id65": identm, "id128": ident128,
        })
    return in_maps


def _host_correction(inputs):
    """Exact batch-independent terms applied on host:
    v-bias through attention (+b_v) and conv; BN shift t_aff; out bias."""
    b_qkv = np.asarray(inputs["b_qkv"], np.float32)
    conv_w = np.asarray(inputs["conv_w"], np.float32)
    conv_b = np.asarray(inputs["conv_b"], np.float32)
    bn_gamma = np.asarray(inputs["bn_gamma"], np.float32)
    bn_beta = np.asarray(inputs["bn_beta"], np.float32)
    bn_mean = np.asarray(inputs["bn_mean"], np.float32)
    bn_var = np.asarray(inputs["bn_var"], np.float32)
    w_out = np.asarray(inputs["w_out"], np.float32)
    b_out = np.asarray(inputs["b_out"], np.float32)

    s = bn_gamma / np.sqrt(bn_var + BN_EPS)
    t_aff = (conv_b - bn_mean) * s + bn_beta
    b_v = b_qkv[2 * DIM:]
    bimg = b_v.reshape(HEADS, DH)
    pad = np.zeros((HEADS + 2, DH + 2), np.float32)
    pad[1:-1, 1:-1] = bimg
    wsum = conv_w.sum(1)                                  # [65, 3, 3]
    dconv = np.zeros((N, HEADS, DH), np.float32)
    for ty in range(3):
        for tx in range(3):
            dconv += wsum[:, ty, tx][:, None, None] * \
                pad[ty:ty + HEADS, tx:tx + DH][None, :, :]
    dres = b_v[None, :] + (dconv * s[:, None, None]).reshape(N, DIM) \
        + t_aff[:, None]
    return dres @ w_out.T + b_out[None, :]                # [N, DIM]


def kernel(x, w_qkv, b_qkv, w_out, b_out, conv_w, conv_b,
           bn_gamma, bn_beta, bn_mean, bn_var):
    import concourse.bass as bass
    import concourse.bacc as bacc
    import concourse.mybir as mybir
    from concourse.bass_utils import run_bass_kernel_spmd

    inputs = dict(x=x, w_qkv=w_qkv, b_qkv=b_qkv, w_out=w_out, b_out=b_out,
                  conv_w=conv_w, conv_b=conv_b, bn_gamma=bn_gamma,
                  bn_beta=bn_beta, bn_mean=bn_mean, bn_var=bn_var)

    nc = bacc.Bacc()
    _build(nc, mybir, bass)
    nc.finalize()

    in_maps = _prepare_inputs(inputs)
    res = run_bass_kernel_spmd(nc, in_maps, list(range(NCORES)))
    outs = [np.ascontiguousarray(res.results[c]["out"].T)     # [2080, 1024]
            for c in range(NCORES)]
    full = np.concatenate(outs, axis=0).reshape(B, N, DIM)
    dout = _host_correction(inputs)
    return full + dout[None, :, :]
